# revision 1
# baseline (speedup 1.0000x reference)
"""Grouped SwiGLU MoE (M=8192, K=2048, N=1024, E=16, top-2) on 8 TRN2 cores.

Strategy: expert-parallel. Host sorts the M*top_k expanded token slots by
expert, gathers+transposes the activations per core (core c owns experts
2c, 2c+1; each expert segment is zero-padded to a static capacity Ce),
and pre-transposes the three weight tensors to bf16. Each core runs a
dense per-expert GEMM chain:

    H^T[n, m] = silu(Wg^T-contract) * (Wu^T-contract)     (PSUM f32, bf16 out)
    out[m, k] = gate_m * (H^T.T @ Wd^T)                   (per-row scale fused)

The host combine is two pure row-gathers + one add (each token appears in
exactly top_k=2 expanded slots).
"""

import numpy as np
import ml_dtypes

import concourse.bass as bass  # noqa: F401  (engine namespace comes via nc)
import concourse.mybir as mybir
import concourse.tile as tile
from concourse import bacc, bass_utils

M, K, N, E, TOPK = 8192, 2048, 1024, 16, 2
NCORES = 8
EPC = E // NCORES  # experts per core
P = 128
KT = K // P   # 16 k-tiles
NT = N // P   # 8 n-tiles
KKC = K // 512  # 4 output column chunks

BF16 = mybir.dt.bfloat16
F32 = mybir.dt.float32
NP_BF16 = ml_dtypes.bfloat16

# Set by a driving harness to collect a profile; read back via LAST_RESULT.
TRACE = False
LAST_RESULT = None

_compiled = {}


def _chunks(Ce):
    out = []
    m0 = 0
    while m0 < Ce:
        f = min(512, Ce - m0)
        out.append((m0, f))
        m0 += f
    return out


def _build(Ce):
    CT = EPC * Ce
    nc = bacc.Bacc()
    xt = nc.dram_tensor("xt", [K, CT], BF16, kind="ExternalInput")
    wg = nc.dram_tensor("wg", [EPC, K, N], BF16, kind="ExternalInput")
    wu = nc.dram_tensor("wu", [EPC, K, N], BF16, kind="ExternalInput")
    wd = nc.dram_tensor("wd", [EPC, N, K], BF16, kind="ExternalInput")
    gv = nc.dram_tensor("gv", [CT, 1], F32, kind="ExternalInput")
    out = nc.dram_tensor("out", [CT, K], F32, kind="ExternalOutput")

    xt_p = xt.rearrange("(kt p) c -> p kt c", p=P)      # [128, KT, CT]
    gv_p = gv.rearrange("(t p) one -> p t one", p=P)    # [128, CT//P, 1]

    with tile.TileContext(nc) as tc:
        with (
            tc.tile_pool(name="wpool", bufs=1) as wpool,
            tc.tile_pool(name="xpool", bufs=2) as xpool,
            tc.tile_pool(name="hpool", bufs=2) as hpool,
            tc.tile_pool(name="spool", bufs=3) as spool,
            tc.tile_pool(name="opool", bufs=2) as opool,
            tc.tile_pool(name="gpool", bufs=2) as gpool,
            tc.tile_pool(name="psum", bufs=2, space="PSUM") as psum,
        ):
            for e in range(EPC):
                wg_sb = wpool.tile([P, KT * N], BF16, tag="wg")
                wu_sb = wpool.tile([P, KT * N], BF16, tag="wu")
                wd_sb = wpool.tile([P, NT * K], BF16, tag="wd")
                nc.sync.dma_start(
                    out=wg_sb[:].rearrange("p (kt n) -> p kt n", n=N),
                    in_=wg[e].rearrange("(kt p) n -> p kt n", p=P),
                )
                nc.sync.dma_start(
                    out=wu_sb[:].rearrange("p (kt n) -> p kt n", n=N),
                    in_=wu[e].rearrange("(kt p) n -> p kt n", p=P),
                )
                nc.sync.dma_start(
                    out=wd_sb[:].rearrange("p (nt k) -> p nt k", k=K),
                    in_=wd[e].rearrange("(nt p) k -> p nt k", p=P),
                )
                for m0, F in _chunks(Ce):
                    col0 = e * Ce + m0
                    xt_sb = xpool.tile([P, KT * F], BF16, tag="xt")
                    nc.sync.dma_start(
                        out=xt_sb[:].rearrange("p (kt f) -> p kt f", f=F),
                        in_=xt_p[:, :, col0:col0 + F],
                    )
                    ht_sb = hpool.tile([P, NT * F], BF16, tag="ht")
                    for nt in range(NT):
                        pg = psum.tile([P, F], F32, tag="pg")
                        pu = psum.tile([P, F], F32, tag="pu")
                        for kt in range(KT):
                            nc.tensor.matmul(
                                pg[:],
                                wg_sb[:, kt * N + nt * P: kt * N + nt * P + P],
                                xt_sb[:, kt * F: (kt + 1) * F],
                                start=(kt == 0),
                                stop=(kt == KT - 1),
                            )
                        for kt in range(KT):
                            nc.tensor.matmul(
                                pu[:],
                                wu_sb[:, kt * N + nt * P: kt * N + nt * P + P],
                                xt_sb[:, kt * F: (kt + 1) * F],
                                start=(kt == 0),
                                stop=(kt == KT - 1),
                            )
                        hg = spool.tile([P, F], F32, tag="hg")
                        nc.scalar.activation(
                            hg[:], pg[:], mybir.ActivationFunctionType.Silu
                        )
                        nc.vector.tensor_tensor(
                            ht_sb[:, nt * F: (nt + 1) * F],
                            hg[:],
                            pu[:],
                            mybir.AluOpType.mult,
                        )
                    for ms in range(F // P):
                        tidx = (col0 + ms * P) // P
                        gvt = gpool.tile([P, 1], F32, tag="gv")
                        nc.sync.dma_start(out=gvt[:], in_=gv_p[:, tidx, :])
                        ot = opool.tile([P, K], F32, tag="ot")
                        for kc in range(KKC):
                            po = psum.tile([P, 512], F32, tag="po")
                            for nt in range(NT):
                                nc.tensor.matmul(
                                    po[:],
                                    ht_sb[:, nt * F + ms * P: nt * F + ms * P + P],
                                    wd_sb[:, nt * K + kc * 512: nt * K + (kc + 1) * 512],
                                    start=(nt == 0),
                                    stop=(nt == NT - 1),
                                )
                            nc.vector.tensor_scalar_mul(
                                ot[:, kc * 512:(kc + 1) * 512], po[:], gvt[:, :1]
                            )
                        nc.sync.dma_start(
                            out=out[col0 + ms * P: col0 + ms * P + P, :], in_=ot[:]
                        )
    nc.compile()
    return nc


def _get(Ce):
    if Ce not in _compiled:
        _compiled[Ce] = _build(Ce)
    return _compiled[Ce]


def kernel(flat_h, flat_idx, flat_gate, gate_weight, up_weight, down_weight):
    global LAST_RESULT
    eid = np.asarray(flat_idx).reshape(-1).astype(np.int64)
    gvals = np.asarray(flat_gate).reshape(-1).astype(np.float32)
    perm = np.argsort(eid, kind="stable")
    cnt = np.bincount(eid, minlength=E)
    offs = np.concatenate([[0], np.cumsum(cnt)])
    Ce = int(max(np.ceil(cnt.max() / P) * P, P))
    CT = EPC * Ce
    nc = _get(Ce)

    Xb = np.asarray(flat_h).astype(NP_BF16)
    wgT = gate_weight.transpose(0, 2, 1).astype(NP_BF16)  # (E, K, N)
    wuT = up_weight.transpose(0, 2, 1).astype(NP_BF16)    # (E, K, N)
    wdT = down_weight.transpose(0, 2, 1).astype(NP_BF16)  # (E, N, K)
    wgT = np.ascontiguousarray(wgT)
    wuT = np.ascontiguousarray(wuT)
    wdT = np.ascontiguousarray(wdT)

    colmap = np.zeros(M * TOPK, dtype=np.int64)
    in_maps = []
    for c in range(NCORES):
        xt = np.zeros((K, CT), dtype=NP_BF16)
        gvc = np.zeros((CT, 1), dtype=np.float32)
        for j in range(EPC):
            e = EPC * c + j
            rows = perm[offs[e]:offs[e + 1]]
            n_e = len(rows)
            xt[:, j * Ce: j * Ce + n_e] = Xb[rows // TOPK].T
            gvc[j * Ce: j * Ce + n_e, 0] = gvals[rows]
            colmap[rows] = c * CT + j * Ce + np.arange(n_e)
        in_maps.append(
            {
                "xt": xt,
                "wg": np.ascontiguousarray(wgT[EPC * c: EPC * (c + 1)]),
                "wu": np.ascontiguousarray(wuT[EPC * c: EPC * (c + 1)]),
                "wd": np.ascontiguousarray(wdT[EPC * c: EPC * (c + 1)]),
                "gv": gvc,
            }
        )

    res = bass_utils.run_bass_kernel_spmd(
        nc, in_maps, core_ids=list(range(NCORES)), trace=TRACE
    )
    LAST_RESULT = res
    Y = np.concatenate([np.asarray(res.results[c]["out"]) for c in range(NCORES)], axis=0)
    out = Y[colmap[0::2]] + Y[colmap[1::2]]
    return np.ascontiguousarray(out, dtype=np.float32)


# revision 7
# speedup vs baseline: 1.0149x; 1.0149x over previous
"""Grouped SwiGLU MoE (M=8192, K=2048, N=1024, E=16, top-2) on 8 TRN2 cores.

Strategy: expert-parallel. Host sorts the M*top_k expanded token slots by
expert, gathers+transposes the activations per core (core c owns experts
2c, 2c+1; each expert segment is zero-padded to a static capacity Ce),
and pre-transposes the three weight tensors to bf16. Each core runs a
dense per-expert GEMM chain:

    H^T[n, m] = silu(Wg^T-contract) * (Wu^T-contract)     (PSUM f32, bf16 out)
    out[m, k] = gate_m * (H^T.T @ Wd^T)                   (per-row scale fused)

The host combine is two pure row-gathers + one add (each token appears in
exactly top_k=2 expanded slots).
"""

import numpy as np
import ml_dtypes

import concourse.bass as bass  # noqa: F401  (engine namespace comes via nc)
import concourse.mybir as mybir
import concourse.tile as tile
from concourse import bacc, bass_utils

M, K, N, E, TOPK = 8192, 2048, 1024, 16, 2
NCORES = 8
EPC = E // NCORES  # experts per core
P = 128
KT = K // P   # 16 k-tiles
NT = N // P   # 8 n-tiles
KKC = K // 512  # 4 output column chunks

BF16 = mybir.dt.bfloat16
F32 = mybir.dt.float32
NP_BF16 = ml_dtypes.bfloat16

# Set by a driving harness to collect a profile; read back via LAST_RESULT.
TRACE = False
LAST_RESULT = None

_compiled = {}


def _chunks(Ce):
    out = []
    m0 = 0
    while m0 < Ce:
        f = min(512, Ce - m0)
        out.append((m0, f))
        m0 += f
    return out


def _build(Ce):
    CT = EPC * Ce
    nc = bacc.Bacc()
    xt = nc.dram_tensor("xt", [K, CT], BF16, kind="ExternalInput")
    wg = nc.dram_tensor("wg", [EPC, K, N], BF16, kind="ExternalInput")
    wu = nc.dram_tensor("wu", [EPC, K, N], BF16, kind="ExternalInput")
    wd = nc.dram_tensor("wd", [EPC, N, K], BF16, kind="ExternalInput")
    gv = nc.dram_tensor("gv", [CT], F32, kind="ExternalInput")
    out = nc.dram_tensor("out", [CT, K], F32, kind="ExternalOutput")

    xt_p = xt.rearrange("(kt p) c -> p kt c", p=P)      # [128, KT, CT]
    gv_p = gv.rearrange("(t p) -> p t", p=P)            # [128, CT//P]

    with tile.TileContext(nc) as tc:
        with (
            tc.tile_pool(name="wpool", bufs=1) as wpool,
            tc.tile_pool(name="xpool", bufs=2) as xpool,
            tc.tile_pool(name="hpool", bufs=2) as hpool,
            tc.tile_pool(name="spool", bufs=3) as spool,
            tc.tile_pool(name="opool", bufs=2) as opool,
            tc.tile_pool(name="gpool", bufs=2) as gpool,
            tc.tile_pool(name="psum", bufs=2, space="PSUM") as psum,
        ):
            gv_sb = gpool.tile([P, CT // P], F32, tag="gv")
            nc.sync.dma_start(out=gv_sb[:], in_=gv_p[:])
            for e in range(EPC):
                wg_sb = wpool.tile([P, KT * N], BF16, tag="wg")
                wu_sb = wpool.tile([P, KT * N], BF16, tag="wu")
                wd_sb = wpool.tile([P, NT * K], BF16, tag="wd")
                wg_d = wg[e].rearrange("(kt p) n -> p kt n", p=P)
                wu_d = wu[e].rearrange("(kt p) n -> p kt n", p=P)
                wd_d = wd[e].rearrange("(nt p) k -> p nt k", p=P)
                for kt in range(KT):
                    nc.sync.dma_start(
                        out=wg_sb[:, kt * N:(kt + 1) * N], in_=wg_d[:, kt, :]
                    )
                for kt in range(KT):
                    nc.sync.dma_start(
                        out=wu_sb[:, kt * N:(kt + 1) * N], in_=wu_d[:, kt, :]
                    )
                for nt in range(NT):
                    nc.sync.dma_start(
                        out=wd_sb[:, nt * K:(nt + 1) * K], in_=wd_d[:, nt, :]
                    )
                for m0, F in _chunks(Ce):
                    col0 = e * Ce + m0
                    xt_sb = xpool.tile([P, KT * F], BF16, tag="xt")
                    for kt in range(KT):
                        nc.sync.dma_start(
                            out=xt_sb[:, kt * F:(kt + 1) * F],
                            in_=xt_p[:, kt, col0:col0 + F],
                        )
                    ht_sb = hpool.tile([P, NT * F], BF16, tag="ht")
                    for nt in range(NT):
                        pg = psum.tile([P, F], F32, tag="pg")
                        pu = psum.tile([P, F], F32, tag="pu")
                        for kt in range(KT):
                            nc.tensor.matmul(
                                pg[:],
                                wg_sb[:, kt * N + nt * P: kt * N + nt * P + P],
                                xt_sb[:, kt * F: (kt + 1) * F],
                                start=(kt == 0),
                                stop=(kt == KT - 1),
                            )
                        for kt in range(KT):
                            nc.tensor.matmul(
                                pu[:],
                                wu_sb[:, kt * N + nt * P: kt * N + nt * P + P],
                                xt_sb[:, kt * F: (kt + 1) * F],
                                start=(kt == 0),
                                stop=(kt == KT - 1),
                            )
                        hg = spool.tile([P, F], F32, tag="hg")
                        nc.scalar.activation(
                            hg[:], pg[:], mybir.ActivationFunctionType.Silu
                        )
                        nc.vector.tensor_tensor(
                            ht_sb[:, nt * F: (nt + 1) * F],
                            hg[:],
                            pu[:],
                            mybir.AluOpType.mult,
                        )
                    for ms in range(F // P):
                        tidx = (col0 + ms * P) // P
                        ot = opool.tile([P, K], F32, tag="ot")
                        for kc in range(KKC):
                            po = psum.tile([P, 512], F32, tag="po")
                            for nt in range(NT):
                                nc.tensor.matmul(
                                    po[:],
                                    ht_sb[:, nt * F + ms * P: nt * F + ms * P + P],
                                    wd_sb[:, nt * K + kc * 512: nt * K + (kc + 1) * 512],
                                    start=(nt == 0),
                                    stop=(nt == NT - 1),
                                )
                            nc.vector.tensor_scalar_mul(
                                ot[:, kc * 512:(kc + 1) * 512],
                                po[:],
                                gv_sb[:, tidx:tidx + 1],
                            )
                        nc.sync.dma_start(
                            out=out[col0 + ms * P: col0 + ms * P + P, :], in_=ot[:]
                        )
    nc.compile()
    return nc


def _get(Ce):
    if Ce not in _compiled:
        _compiled[Ce] = _build(Ce)
    return _compiled[Ce]


def kernel(flat_h, flat_idx, flat_gate, gate_weight, up_weight, down_weight):
    global LAST_RESULT
    eid = np.asarray(flat_idx).reshape(-1).astype(np.int64)
    gvals = np.asarray(flat_gate).reshape(-1).astype(np.float32)
    perm = np.argsort(eid, kind="stable")
    cnt = np.bincount(eid, minlength=E)
    offs = np.concatenate([[0], np.cumsum(cnt)])
    Ce = int(max(np.ceil(cnt.max() / P) * P, P))
    CT = EPC * Ce
    nc = _get(Ce)

    Xb = np.asarray(flat_h).astype(NP_BF16)
    wgT = gate_weight.transpose(0, 2, 1).astype(NP_BF16)  # (E, K, N)
    wuT = up_weight.transpose(0, 2, 1).astype(NP_BF16)    # (E, K, N)
    wdT = down_weight.transpose(0, 2, 1).astype(NP_BF16)  # (E, N, K)
    wgT = np.ascontiguousarray(wgT)
    wuT = np.ascontiguousarray(wuT)
    wdT = np.ascontiguousarray(wdT)

    colmap = np.zeros(M * TOPK, dtype=np.int64)
    in_maps = []
    for c in range(NCORES):
        xt = np.zeros((K, CT), dtype=NP_BF16)
        gvc = np.zeros((CT,), dtype=np.float32)
        for j in range(EPC):
            e = EPC * c + j
            rows = perm[offs[e]:offs[e + 1]]
            n_e = len(rows)
            xt[:, j * Ce: j * Ce + n_e] = Xb[rows // TOPK].T
            gvc[j * Ce: j * Ce + n_e] = gvals[rows]
            colmap[rows] = c * CT + j * Ce + np.arange(n_e)
        in_maps.append(
            {
                "xt": xt,
                "wg": np.ascontiguousarray(wgT[EPC * c: EPC * (c + 1)]),
                "wu": np.ascontiguousarray(wuT[EPC * c: EPC * (c + 1)]),
                "wd": np.ascontiguousarray(wdT[EPC * c: EPC * (c + 1)]),
                "gv": gvc,
            }
        )

    res = bass_utils.run_bass_kernel_spmd(
        nc, in_maps, core_ids=list(range(NCORES)), trace=TRACE
    )
    LAST_RESULT = res
    Y = np.concatenate([np.asarray(res.results[c]["out"]) for c in range(NCORES)], axis=0)
    out = Y[colmap[0::2]] + Y[colmap[1::2]]
    return np.ascontiguousarray(out, dtype=np.float32)


# revision 8
# speedup vs baseline: 1.0603x; 1.0447x over previous
"""Grouped SwiGLU MoE (M=8192, K=2048, N=1024, E=16, top-2) on 8 TRN2 cores.

Strategy: expert-parallel. Host sorts the M*top_k expanded token slots by
expert, gathers+transposes the activations per core (core c owns experts
2c, 2c+1; each expert segment is zero-padded to a static capacity Ce),
and pre-transposes the three weight tensors to bf16. Each core runs a
dense per-expert GEMM chain:

    H^T[n, m] = silu(Wg^T-contract) * (Wu^T-contract)     (PSUM f32, bf16 out)
    out[m, k] = gate_m * (H^T.T @ Wd^T)                   (per-row scale fused)

The host combine is two pure row-gathers + one add (each token appears in
exactly top_k=2 expanded slots).
"""

import numpy as np
import ml_dtypes

import concourse.bass as bass  # noqa: F401  (engine namespace comes via nc)
import concourse.mybir as mybir
import concourse.tile as tile
from concourse import bacc, bass_utils

M, K, N, E, TOPK = 8192, 2048, 1024, 16, 2
NCORES = 8
EPC = E // NCORES  # experts per core
P = 128
KT = K // P   # 16 k-tiles
NT = N // P   # 8 n-tiles
KKC = K // 512  # 4 output column chunks

BF16 = mybir.dt.bfloat16
F32 = mybir.dt.float32
NP_BF16 = ml_dtypes.bfloat16

# Set by a driving harness to collect a profile; read back via LAST_RESULT.
TRACE = False
LAST_RESULT = None

_compiled = {}


def _chunks(Ce):
    out = []
    m0 = 0
    while m0 < Ce:
        f = min(512, Ce - m0)
        out.append((m0, f))
        m0 += f
    return out


def _build(Ce):
    CT = EPC * Ce
    nc = bacc.Bacc()
    xt = nc.dram_tensor("xt", [K, CT], BF16, kind="ExternalInput")
    wg = nc.dram_tensor("wg", [EPC, K, N], BF16, kind="ExternalInput")
    wu = nc.dram_tensor("wu", [EPC, K, N], BF16, kind="ExternalInput")
    wd = nc.dram_tensor("wd", [EPC, N, K], BF16, kind="ExternalInput")
    gv = nc.dram_tensor("gv", [CT], F32, kind="ExternalInput")
    out = nc.dram_tensor("out", [CT, K], F32, kind="ExternalOutput")

    xt_p = xt.rearrange("(kt p) c -> p kt c", p=P)      # [128, KT, CT]
    gv_p = gv.rearrange("(t p) -> p t", p=P)            # [128, CT//P]

    with tile.TileContext(nc) as tc:
        with (
            tc.tile_pool(name="wpool", bufs=1) as wpool,
            tc.tile_pool(name="xpool", bufs=2) as xpool,
            tc.tile_pool(name="hpool", bufs=2) as hpool,
            tc.tile_pool(name="spool", bufs=3) as spool,
            tc.tile_pool(name="opool", bufs=2) as opool,
            tc.tile_pool(name="gpool", bufs=2) as gpool,
            tc.tile_pool(name="psum", bufs=2, space="PSUM") as psum,
        ):
            gv_sb = gpool.tile([P, CT // P], F32, tag="gv")
            nc.sync.dma_start(out=gv_sb[:], in_=gv_p[:])
            for e in range(EPC):
                wg_sb = wpool.tile([P, KT * N], BF16, tag="wg")
                wu_sb = wpool.tile([P, KT * N], BF16, tag="wu")
                wd_sb = wpool.tile([P, NT * K], BF16, tag="wd")
                wg_d = wg[e].rearrange("(kt p) n -> p kt n", p=P)
                wu_d = wu[e].rearrange("(kt p) n -> p kt n", p=P)
                wd_d = wd[e].rearrange("(nt p) k -> p nt k", p=P)
                chunks = _chunks(Ce)
                # First chunk's activations interleave with wg so the first
                # GEMM group can start as soon as the k-blocks land; wu/wd
                # stream behind it under the shadow of G1/G2 compute.
                xt0_sb = xpool.tile([P, KT * chunks[0][1]], BF16, tag="xt")
                F0 = chunks[0][1]
                for kt in range(KT):
                    nc.sync.dma_start(
                        out=wg_sb[:, kt * N:(kt + 1) * N], in_=wg_d[:, kt, :]
                    )
                    nc.sync.dma_start(
                        out=xt0_sb[:, kt * F0:(kt + 1) * F0],
                        in_=xt_p[:, kt, e * Ce:e * Ce + F0],
                    )
                for kt in range(KT):
                    nc.sync.dma_start(
                        out=wu_sb[:, kt * N:(kt + 1) * N], in_=wu_d[:, kt, :]
                    )
                for nt in range(NT):
                    nc.sync.dma_start(
                        out=wd_sb[:, nt * K:(nt + 1) * K], in_=wd_d[:, nt, :]
                    )
                for ci, (m0, F) in enumerate(chunks):
                    col0 = e * Ce + m0
                    if ci == 0:
                        xt_sb = xt0_sb
                    else:
                        xt_sb = xpool.tile([P, KT * F], BF16, tag="xt")
                        for kt in range(KT):
                            nc.sync.dma_start(
                                out=xt_sb[:, kt * F:(kt + 1) * F],
                                in_=xt_p[:, kt, col0:col0 + F],
                            )
                    ht_sb = hpool.tile([P, NT * F], BF16, tag="ht")
                    for nt in range(NT):
                        pg = psum.tile([P, F], F32, tag="pg")
                        pu = psum.tile([P, F], F32, tag="pu")
                        for kt in range(KT):
                            nc.tensor.matmul(
                                pg[:],
                                wg_sb[:, kt * N + nt * P: kt * N + nt * P + P],
                                xt_sb[:, kt * F: (kt + 1) * F],
                                start=(kt == 0),
                                stop=(kt == KT - 1),
                            )
                        for kt in range(KT):
                            nc.tensor.matmul(
                                pu[:],
                                wu_sb[:, kt * N + nt * P: kt * N + nt * P + P],
                                xt_sb[:, kt * F: (kt + 1) * F],
                                start=(kt == 0),
                                stop=(kt == KT - 1),
                            )
                        hg = spool.tile([P, F], F32, tag="hg")
                        nc.scalar.activation(
                            hg[:], pg[:], mybir.ActivationFunctionType.Silu
                        )
                        nc.vector.tensor_tensor(
                            ht_sb[:, nt * F: (nt + 1) * F],
                            hg[:],
                            pu[:],
                            mybir.AluOpType.mult,
                        )
                    for ms in range(F // P):
                        tidx = (col0 + ms * P) // P
                        ot = opool.tile([P, K], F32, tag="ot")
                        for kc in range(KKC):
                            po = psum.tile([P, 512], F32, tag="po")
                            for nt in range(NT):
                                nc.tensor.matmul(
                                    po[:],
                                    ht_sb[:, nt * F + ms * P: nt * F + ms * P + P],
                                    wd_sb[:, nt * K + kc * 512: nt * K + (kc + 1) * 512],
                                    start=(nt == 0),
                                    stop=(nt == NT - 1),
                                )
                            nc.vector.tensor_scalar_mul(
                                ot[:, kc * 512:(kc + 1) * 512],
                                po[:],
                                gv_sb[:, tidx:tidx + 1],
                            )
                        nc.sync.dma_start(
                            out=out[col0 + ms * P: col0 + ms * P + P, :], in_=ot[:]
                        )
    nc.compile()
    return nc


def _get(Ce):
    if Ce not in _compiled:
        _compiled[Ce] = _build(Ce)
    return _compiled[Ce]


def kernel(flat_h, flat_idx, flat_gate, gate_weight, up_weight, down_weight):
    global LAST_RESULT
    eid = np.asarray(flat_idx).reshape(-1).astype(np.int64)
    gvals = np.asarray(flat_gate).reshape(-1).astype(np.float32)
    perm = np.argsort(eid, kind="stable")
    cnt = np.bincount(eid, minlength=E)
    offs = np.concatenate([[0], np.cumsum(cnt)])
    Ce = int(max(np.ceil(cnt.max() / P) * P, P))
    CT = EPC * Ce
    nc = _get(Ce)

    Xb = np.asarray(flat_h).astype(NP_BF16)
    wgT = gate_weight.transpose(0, 2, 1).astype(NP_BF16)  # (E, K, N)
    wuT = up_weight.transpose(0, 2, 1).astype(NP_BF16)    # (E, K, N)
    wdT = down_weight.transpose(0, 2, 1).astype(NP_BF16)  # (E, N, K)
    wgT = np.ascontiguousarray(wgT)
    wuT = np.ascontiguousarray(wuT)
    wdT = np.ascontiguousarray(wdT)

    colmap = np.zeros(M * TOPK, dtype=np.int64)
    in_maps = []
    for c in range(NCORES):
        xt = np.zeros((K, CT), dtype=NP_BF16)
        gvc = np.zeros((CT,), dtype=np.float32)
        for j in range(EPC):
            e = EPC * c + j
            rows = perm[offs[e]:offs[e + 1]]
            n_e = len(rows)
            xt[:, j * Ce: j * Ce + n_e] = Xb[rows // TOPK].T
            gvc[j * Ce: j * Ce + n_e] = gvals[rows]
            colmap[rows] = c * CT + j * Ce + np.arange(n_e)
        in_maps.append(
            {
                "xt": xt,
                "wg": np.ascontiguousarray(wgT[EPC * c: EPC * (c + 1)]),
                "wu": np.ascontiguousarray(wuT[EPC * c: EPC * (c + 1)]),
                "wd": np.ascontiguousarray(wdT[EPC * c: EPC * (c + 1)]),
                "gv": gvc,
            }
        )

    res = bass_utils.run_bass_kernel_spmd(
        nc, in_maps, core_ids=list(range(NCORES)), trace=TRACE
    )
    LAST_RESULT = res
    Y = np.concatenate([np.asarray(res.results[c]["out"]) for c in range(NCORES)], axis=0)
    out = Y[colmap[0::2]] + Y[colmap[1::2]]
    return np.ascontiguousarray(out, dtype=np.float32)


# revision 13
# speedup vs baseline: 1.1085x; 1.0455x over previous
"""Grouped SwiGLU MoE (M=8192, K=2048, N=1024, E=16, top-2) on 8 TRN2 cores.

Strategy: expert-parallel. Host sorts the M*top_k expanded token slots by
expert, gathers+transposes the activations per core (core c owns experts
2c, 2c+1; each expert segment is zero-padded to a static capacity Ce),
and pre-transposes the three weight tensors to bf16. Each core runs a
dense per-expert GEMM chain:

    H^T[n, m] = silu(Wg^T-contract) * (Wu^T-contract)     (PSUM f32, bf16 out)
    out[m, k] = gate_m * (H^T.T @ Wd^T)                   (per-row scale fused)

The host combine is two pure row-gathers + one add (each token appears in
exactly top_k=2 expanded slots).
"""

import numpy as np
import ml_dtypes

import concourse.bass as bass  # noqa: F401  (engine namespace comes via nc)
import concourse.mybir as mybir
import concourse.tile as tile
from concourse import bacc, bass_utils

M, K, N, E, TOPK = 8192, 2048, 1024, 16, 2
NCORES = 8
EPC = E // NCORES  # experts per core
P = 128
KT = K // P   # 16 k-tiles
NT = N // P   # 8 n-tiles
KKC = K // 512  # 4 output column chunks

BF16 = mybir.dt.bfloat16
F32 = mybir.dt.float32
NP_BF16 = ml_dtypes.bfloat16

# Set by a driving harness to collect a profile; read back via LAST_RESULT.
TRACE = False
LAST_RESULT = None

_compiled = {}


def _chunks(Ce):
    out = []
    m0 = 0
    while m0 < Ce:
        f = min(512, Ce - m0)
        out.append((m0, f))
        m0 += f
    return out


def _build(Ce):
    CT = EPC * Ce
    nc = bacc.Bacc()
    xt = nc.dram_tensor("xt", [K, CT], BF16, kind="ExternalInput")
    wg = nc.dram_tensor("wg", [EPC, K, N], BF16, kind="ExternalInput")
    wu = nc.dram_tensor("wu", [EPC, K, N], BF16, kind="ExternalInput")
    wd = nc.dram_tensor("wd", [EPC, N, K], BF16, kind="ExternalInput")
    gv = nc.dram_tensor("gv", [CT], F32, kind="ExternalInput")
    out = nc.dram_tensor("out", [CT, K], F32, kind="ExternalOutput")

    xt_p = xt.rearrange("(kt p) c -> p kt c", p=P)      # [128, KT, CT]
    gv_p = gv.rearrange("(t p) -> p t", p=P)            # [128, CT//P]

    with tile.TileContext(nc) as tc:
        with (
            tc.tile_pool(name="wpool", bufs=1) as wpool,
            tc.tile_pool(name="xpool", bufs=2) as xpool,
            tc.tile_pool(name="hpool", bufs=3) as hpool,
            tc.tile_pool(name="spool", bufs=8) as spool,
            tc.tile_pool(name="opool", bufs=2) as opool,
            tc.tile_pool(name="gpool", bufs=1) as gpool,
            tc.tile_pool(name="psum", bufs=8, space="PSUM") as psum,
        ):

            def g1g2(wg_sb, wu_sb, xt_sb, F, kt_outer):
                """Compute H^T for one m-chunk; returns the bf16 ht tile."""
                ht_sb = hpool.tile([P, NT * F], BF16, tag="ht")
                if kt_outer:
                    # All NT accumulation groups open at once so the PE can
                    # consume each wg/xt k-block the moment its DMA lands
                    # (startup: weights are still streaming in from HBM).
                    pgs = [psum.tile([P, F], F32, tag="ps", name=f"pg{nt}") for nt in range(NT)]
                    for kt in range(KT):
                        for nt in range(NT):
                            nc.tensor.matmul(
                                pgs[nt][:],
                                wg_sb[:, kt * N + nt * P: kt * N + nt * P + P],
                                xt_sb[:, kt * F: (kt + 1) * F],
                                start=(kt == 0),
                                stop=(kt == KT - 1),
                            )
                    hgs = []
                    for nt in range(NT):
                        hg = spool.tile([P, F], F32, tag="hg")
                        nc.scalar.activation(
                            hg[:], pgs[nt][:], mybir.ActivationFunctionType.Silu
                        )
                        hgs.append(hg)
                    pus = [psum.tile([P, F], F32, tag="ps", name=f"pu{nt}") for nt in range(NT)]
                    for kt in range(KT):
                        for nt in range(NT):
                            nc.tensor.matmul(
                                pus[nt][:],
                                wu_sb[:, kt * N + nt * P: kt * N + nt * P + P],
                                xt_sb[:, kt * F: (kt + 1) * F],
                                start=(kt == 0),
                                stop=(kt == KT - 1),
                            )
                    for nt in range(NT):
                        nc.vector.tensor_tensor(
                            ht_sb[:, nt * F: (nt + 1) * F],
                            hgs[nt][:],
                            pus[nt][:],
                            mybir.AluOpType.mult,
                        )
                else:
                    for nt in range(NT):
                        pg = psum.tile([P, F], F32, tag="ps")
                        pu = psum.tile([P, F], F32, tag="ps")
                        for kt in range(KT):
                            nc.tensor.matmul(
                                pg[:],
                                wg_sb[:, kt * N + nt * P: kt * N + nt * P + P],
                                xt_sb[:, kt * F: (kt + 1) * F],
                                start=(kt == 0),
                                stop=(kt == KT - 1),
                            )
                        for kt in range(KT):
                            nc.tensor.matmul(
                                pu[:],
                                wu_sb[:, kt * N + nt * P: kt * N + nt * P + P],
                                xt_sb[:, kt * F: (kt + 1) * F],
                                start=(kt == 0),
                                stop=(kt == KT - 1),
                            )
                        hg = spool.tile([P, F], F32, tag="hg")
                        nc.scalar.activation(
                            hg[:], pg[:], mybir.ActivationFunctionType.Silu
                        )
                        nc.vector.tensor_tensor(
                            ht_sb[:, nt * F: (nt + 1) * F],
                            hg[:],
                            pu[:],
                            mybir.AluOpType.mult,
                        )
                return ht_sb

            def g3(wd_sb, ht_sb, F, col0):
                for ms in range(F // P):
                    tidx = (col0 + ms * P) // P
                    ot = opool.tile([P, K], F32, tag="ot")
                    for kc in range(KKC):
                        po = psum.tile([P, 512], F32, tag="ps")
                        for nt in range(NT):
                            nc.tensor.matmul(
                                po[:],
                                ht_sb[:, nt * F + ms * P: nt * F + ms * P + P],
                                wd_sb[:, nt * K + kc * 512: nt * K + (kc + 1) * 512],
                                start=(nt == 0),
                                stop=(nt == NT - 1),
                            )
                        nc.vector.tensor_scalar_mul(
                            ot[:, kc * 512:(kc + 1) * 512],
                            po[:],
                            gv_sb[:, tidx:tidx + 1],
                        )
                    nc.sync.dma_start(
                        out=out[col0 + ms * P: col0 + ms * P + P, :], in_=ot[:]
                    )

            gv_sb = gpool.tile([P, CT // P], F32, tag="gv")
            nc.sync.dma_start(out=gv_sb[:], in_=gv_p[:])
            chunks = _chunks(Ce)
            # G3 is deferred one chunk (software pipeline): its matmuls fill
            # the PE bubble while the next chunk's silu/mul chain drains, and
            # the next expert's weight DMAs hide under the deferred G3 work.
            pending = None
            for e in range(EPC):
                wg_sb = wpool.tile([P, KT * N], BF16, tag="wg")
                wu_sb = wpool.tile([P, KT * N], BF16, tag="wu")
                wd_sb = wpool.tile([P, NT * K], BF16, tag="wd")
                wg_d = wg[e].rearrange("(kt p) n -> p kt n", p=P)
                wu_d = wu[e].rearrange("(kt p) n -> p kt n", p=P)
                wd_d = wd[e].rearrange("(nt p) k -> p nt k", p=P)
                # First chunk's activations interleave with wg so the first
                # GEMM group can start as soon as the k-blocks land; wu/wd
                # stream behind it under the shadow of G1/G2 compute.
                F0 = chunks[0][1]
                xt0_sb = xpool.tile([P, KT * F0], BF16, tag="xt")
                for kt in range(KT):
                    nc.sync.dma_start(
                        out=wg_sb[:, kt * N:(kt + 1) * N], in_=wg_d[:, kt, :]
                    )
                    nc.sync.dma_start(
                        out=xt0_sb[:, kt * F0:(kt + 1) * F0],
                        in_=xt_p[:, kt, e * Ce:e * Ce + F0],
                    )
                for kt in range(KT):
                    nc.sync.dma_start(
                        out=wu_sb[:, kt * N:(kt + 1) * N], in_=wu_d[:, kt, :]
                    )
                for nt in range(NT):
                    nc.sync.dma_start(
                        out=wd_sb[:, nt * K:(nt + 1) * K], in_=wd_d[:, nt, :]
                    )
                if pending is not None:
                    # Flush after this expert's weight-DMA emission (so the
                    # transfers start under this G3's compute cover) but
                    # before its first GEMM group: the PE is in-order, and
                    # this G3's work is ready now while the new expert's
                    # matmuls would head-of-line block on the weight DMAs.
                    g3(*pending)
                    pending = None
                for ci, (m0, F) in enumerate(chunks):
                    col0 = e * Ce + m0
                    if ci == 0:
                        xt_sb = xt0_sb
                    else:
                        xt_sb = xpool.tile([P, KT * F], BF16, tag="xt")
                        for kt in range(KT):
                            nc.sync.dma_start(
                                out=xt_sb[:, kt * F:(kt + 1) * F],
                                in_=xt_p[:, kt, col0:col0 + F],
                            )
                    ht_sb = g1g2(wg_sb, wu_sb, xt_sb, F, kt_outer=(e == 0 and ci == 0))
                    if pending is not None:
                        g3(*pending)
                    pending = (wd_sb, ht_sb, F, col0)
            if pending is not None:
                g3(*pending)
    nc.compile()
    return nc


def _get(Ce):
    if Ce not in _compiled:
        _compiled[Ce] = _build(Ce)
    return _compiled[Ce]


def kernel(flat_h, flat_idx, flat_gate, gate_weight, up_weight, down_weight):
    global LAST_RESULT
    eid = np.asarray(flat_idx).reshape(-1).astype(np.int64)
    gvals = np.asarray(flat_gate).reshape(-1).astype(np.float32)
    perm = np.argsort(eid, kind="stable")
    cnt = np.bincount(eid, minlength=E)
    offs = np.concatenate([[0], np.cumsum(cnt)])
    Ce = int(max(np.ceil(cnt.max() / P) * P, P))
    CT = EPC * Ce
    nc = _get(Ce)

    Xb = np.asarray(flat_h).astype(NP_BF16)
    wgT = gate_weight.transpose(0, 2, 1).astype(NP_BF16)  # (E, K, N)
    wuT = up_weight.transpose(0, 2, 1).astype(NP_BF16)    # (E, K, N)
    wdT = down_weight.transpose(0, 2, 1).astype(NP_BF16)  # (E, N, K)
    wgT = np.ascontiguousarray(wgT)
    wuT = np.ascontiguousarray(wuT)
    wdT = np.ascontiguousarray(wdT)

    colmap = np.zeros(M * TOPK, dtype=np.int64)
    in_maps = []
    for c in range(NCORES):
        xt = np.zeros((K, CT), dtype=NP_BF16)
        gvc = np.zeros((CT,), dtype=np.float32)
        for j in range(EPC):
            e = EPC * c + j
            rows = perm[offs[e]:offs[e + 1]]
            n_e = len(rows)
            xt[:, j * Ce: j * Ce + n_e] = Xb[rows // TOPK].T
            gvc[j * Ce: j * Ce + n_e] = gvals[rows]
            colmap[rows] = c * CT + j * Ce + np.arange(n_e)
        in_maps.append(
            {
                "xt": xt,
                "wg": np.ascontiguousarray(wgT[EPC * c: EPC * (c + 1)]),
                "wu": np.ascontiguousarray(wuT[EPC * c: EPC * (c + 1)]),
                "wd": np.ascontiguousarray(wdT[EPC * c: EPC * (c + 1)]),
                "gv": gvc,
            }
        )

    res = bass_utils.run_bass_kernel_spmd(
        nc, in_maps, core_ids=list(range(NCORES)), trace=TRACE
    )
    LAST_RESULT = res
    Y = np.concatenate([np.asarray(res.results[c]["out"]) for c in range(NCORES)], axis=0)
    out = Y[colmap[0::2]] + Y[colmap[1::2]]
    return np.ascontiguousarray(out, dtype=np.float32)


# revision 17
# speedup vs baseline: 1.1185x; 1.0091x over previous
"""Grouped SwiGLU MoE (M=8192, K=2048, N=1024, E=16, top-2) on 8 TRN2 cores.

Strategy: expert-parallel. Host sorts the M*top_k expanded token slots by
expert, gathers+transposes the activations per core (core c owns experts
2c, 2c+1; each expert segment is zero-padded to a static capacity Ce),
and pre-transposes the three weight tensors to bf16. Each core runs a
dense per-expert GEMM chain:

    H^T[n, m] = silu(Wg^T-contract) * (Wu^T-contract)     (PSUM f32, bf16 out)
    out[m, k] = gate_m * (H^T.T @ Wd^T)                   (per-row scale fused)

The host combine is two pure row-gathers + one add (each token appears in
exactly top_k=2 expanded slots).
"""

import numpy as np
import ml_dtypes

import concourse.bass as bass  # noqa: F401  (engine namespace comes via nc)
import concourse.mybir as mybir
import concourse.tile as tile
from concourse import bacc, bass_utils

M, K, N, E, TOPK = 8192, 2048, 1024, 16, 2
NCORES = 8
EPC = E // NCORES  # experts per core
P = 128
KT = K // P   # 16 k-tiles
NT = N // P   # 8 n-tiles
KKC = K // 512  # 4 output column chunks

BF16 = mybir.dt.bfloat16
F32 = mybir.dt.float32
NP_BF16 = ml_dtypes.bfloat16

# Set by a driving harness to collect a profile; read back via LAST_RESULT.
TRACE = False
LAST_RESULT = None

_compiled = {}


def _chunks(Ce):
    out = []
    m0 = 0
    while m0 < Ce:
        f = min(512, Ce - m0)
        out.append((m0, f))
        m0 += f
    return out


def _build(Ce):
    CT = EPC * Ce
    nc = bacc.Bacc()
    xt = nc.dram_tensor("xt", [K, CT], BF16, kind="ExternalInput")
    wg = nc.dram_tensor("wg", [EPC, K, N], BF16, kind="ExternalInput")
    wu = nc.dram_tensor("wu", [EPC, K, N], BF16, kind="ExternalInput")
    wd = nc.dram_tensor("wd", [EPC, N, K], BF16, kind="ExternalInput")
    gv = nc.dram_tensor("gv", [CT], F32, kind="ExternalInput")
    out = nc.dram_tensor("out", [CT, K], F32, kind="ExternalOutput")

    xt_p = xt.rearrange("(kt p) c -> p kt c", p=P)      # [128, KT, CT]
    gv_p = gv.rearrange("(t p) -> p t", p=P)            # [128, CT//P]

    with tile.TileContext(nc) as tc:
        with (
            tc.tile_pool(name="wpool", bufs=1) as wpool,
            tc.tile_pool(name="xpool", bufs=2) as xpool,
            tc.tile_pool(name="hpool", bufs=3) as hpool,
            tc.tile_pool(name="spool", bufs=8) as spool,
            tc.tile_pool(name="opool", bufs=2) as opool,
            tc.tile_pool(name="gpool", bufs=1) as gpool,
            tc.tile_pool(name="psum", bufs=8, space="PSUM") as psum,
        ):

            def g1g2(wg_sb, wu_sb, xt_sb, F, kt_outer):
                """Compute H^T for one m-chunk; returns the bf16 ht tile."""
                ht_sb = hpool.tile([P, NT * F], BF16, tag="ht")
                if kt_outer:
                    # All NT accumulation groups open at once so the PE can
                    # consume each wg/xt k-block the moment its DMA lands
                    # (startup: weights are still streaming in from HBM).
                    pgs = [psum.tile([P, F], F32, tag="ps", name=f"pg{nt}") for nt in range(NT)]
                    # Warm the PE HAM clock gate during the initial weight
                    # stream: ~3.4us of junk matmuls on a memset scratch tile
                    # so the first real groups run at 2.4 GHz, not 1.2.
                    scr = xpool.tile([P, P], BF16, tag="scr", name="scr")
                    nc.vector.memset(scr[:], 0.0)
                    for i in range(32):
                        nc.tensor.matmul(
                            pgs[0][:, :P],
                            scr[:],
                            scr[:],
                            start=(i == 0),
                            stop=(i == 31),
                        )
                    for kt in range(KT):
                        for nt in range(NT):
                            nc.tensor.matmul(
                                pgs[nt][:],
                                wg_sb[:, kt * N + nt * P: kt * N + nt * P + P],
                                xt_sb[:, kt * F: (kt + 1) * F],
                                start=(kt == 0),
                                stop=(kt == KT - 1),
                            )
                    hgs = []
                    for nt in range(NT):
                        hg = spool.tile([P, F], F32, tag="hg")
                        nc.scalar.activation(
                            hg[:], pgs[nt][:], mybir.ActivationFunctionType.Silu
                        )
                        hgs.append(hg)
                    pus = [psum.tile([P, F], F32, tag="ps", name=f"pu{nt}") for nt in range(NT)]
                    for kt in range(KT):
                        for nt in range(NT):
                            nc.tensor.matmul(
                                pus[nt][:],
                                wu_sb[:, kt * N + nt * P: kt * N + nt * P + P],
                                xt_sb[:, kt * F: (kt + 1) * F],
                                start=(kt == 0),
                                stop=(kt == KT - 1),
                            )
                    for nt in range(NT):
                        nc.vector.tensor_tensor(
                            ht_sb[:, nt * F: (nt + 1) * F],
                            hgs[nt][:],
                            pus[nt][:],
                            mybir.AluOpType.mult,
                        )
                else:
                    for nt in range(NT):
                        pg = psum.tile([P, F], F32, tag="ps")
                        pu = psum.tile([P, F], F32, tag="ps")
                        for kt in range(KT):
                            nc.tensor.matmul(
                                pg[:],
                                wg_sb[:, kt * N + nt * P: kt * N + nt * P + P],
                                xt_sb[:, kt * F: (kt + 1) * F],
                                start=(kt == 0),
                                stop=(kt == KT - 1),
                            )
                        for kt in range(KT):
                            nc.tensor.matmul(
                                pu[:],
                                wu_sb[:, kt * N + nt * P: kt * N + nt * P + P],
                                xt_sb[:, kt * F: (kt + 1) * F],
                                start=(kt == 0),
                                stop=(kt == KT - 1),
                            )
                        hg = spool.tile([P, F], F32, tag="hg")
                        nc.scalar.activation(
                            hg[:], pg[:], mybir.ActivationFunctionType.Silu
                        )
                        nc.vector.tensor_tensor(
                            ht_sb[:, nt * F: (nt + 1) * F],
                            hg[:],
                            pu[:],
                            mybir.AluOpType.mult,
                        )
                return ht_sb

            def g3(wd_sb, ht_sb, F, col0):
                for ms in range(F // P):
                    tidx = (col0 + ms * P) // P
                    ot = opool.tile([P, K], F32, tag="ot")
                    for kc in range(KKC):
                        po = psum.tile([P, 512], F32, tag="ps")
                        for nt in range(NT):
                            nc.tensor.matmul(
                                po[:],
                                ht_sb[:, nt * F + ms * P: nt * F + ms * P + P],
                                wd_sb[:, nt * K + kc * 512: nt * K + (kc + 1) * 512],
                                start=(nt == 0),
                                stop=(nt == NT - 1),
                            )
                        nc.vector.tensor_scalar_mul(
                            ot[:, kc * 512:(kc + 1) * 512],
                            po[:],
                            gv_sb[:, tidx:tidx + 1],
                        )
                        nc.sync.dma_start(
                            out=out[col0 + ms * P: col0 + ms * P + P,
                                    kc * 512:(kc + 1) * 512],
                            in_=ot[:, kc * 512:(kc + 1) * 512],
                        )

            gv_sb = gpool.tile([P, CT // P], F32, tag="gv")
            chunks = _chunks(Ce)
            # G3 is deferred one chunk (software pipeline): its matmuls fill
            # the PE bubble while the next chunk's silu/mul chain drains, and
            # the next expert's weight DMAs hide under the deferred G3 work.
            pending = None
            for e in range(EPC):
                wg_sb = wpool.tile([P, KT * N], BF16, tag="wg")
                wu_sb = wpool.tile([P, KT * N], BF16, tag="wu")
                wd_sb = wpool.tile([P, NT * K], BF16, tag="wd")
                wg_d = wg[e].rearrange("(kt p) n -> p kt n", p=P)
                wu_d = wu[e].rearrange("(kt p) n -> p kt n", p=P)
                wd_d = wd[e].rearrange("(nt p) k -> p nt k", p=P)
                # First chunk's activations interleave with wg so the first
                # GEMM group can start as soon as the k-blocks land; wu/wd
                # stream behind it under the shadow of G1/G2 compute.
                F0 = chunks[0][1]
                xt0_sb = xpool.tile([P, KT * F0], BF16, tag="xt")
                for kt in range(KT):
                    nc.sync.dma_start(
                        out=wg_sb[:, kt * N:(kt + 1) * N], in_=wg_d[:, kt, :]
                    )
                    nc.sync.dma_start(
                        out=xt0_sb[:, kt * F0:(kt + 1) * F0],
                        in_=xt_p[:, kt, e * Ce:e * Ce + F0],
                    )
                for kt in range(KT):
                    nc.sync.dma_start(
                        out=wu_sb[:, kt * N:(kt + 1) * N], in_=wu_d[:, kt, :]
                    )
                for nt in range(NT):
                    nc.sync.dma_start(
                        out=wd_sb[:, nt * K:(nt + 1) * K], in_=wd_d[:, nt, :]
                    )
                if e == 0:
                    # Scattered 4-byte gather; emitted after the bulk weight
                    # DMAs so it can't head-of-line block them. First needed
                    # by the (deferred) first G3.
                    nc.sync.dma_start(out=gv_sb[:], in_=gv_p[:])
                if pending is not None:
                    # Flush after this expert's weight-DMA emission (so the
                    # transfers start under this G3's compute cover) but
                    # before its first GEMM group: the PE is in-order, and
                    # this G3's work is ready now while the new expert's
                    # matmuls would head-of-line block on the weight DMAs.
                    g3(*pending)
                    pending = None
                for ci, (m0, F) in enumerate(chunks):
                    col0 = e * Ce + m0
                    if ci == 0:
                        xt_sb = xt0_sb
                    else:
                        xt_sb = xpool.tile([P, KT * F], BF16, tag="xt")
                        for kt in range(KT):
                            nc.sync.dma_start(
                                out=xt_sb[:, kt * F:(kt + 1) * F],
                                in_=xt_p[:, kt, col0:col0 + F],
                            )
                    ht_sb = g1g2(wg_sb, wu_sb, xt_sb, F, kt_outer=(e == 0 and ci == 0))
                    if pending is not None:
                        g3(*pending)
                    pending = (wd_sb, ht_sb, F, col0)
            if pending is not None:
                g3(*pending)
    nc.compile()
    return nc


def _get(Ce):
    if Ce not in _compiled:
        _compiled[Ce] = _build(Ce)
    return _compiled[Ce]


def kernel(flat_h, flat_idx, flat_gate, gate_weight, up_weight, down_weight):
    global LAST_RESULT
    eid = np.asarray(flat_idx).reshape(-1).astype(np.int64)
    gvals = np.asarray(flat_gate).reshape(-1).astype(np.float32)
    perm = np.argsort(eid, kind="stable")
    cnt = np.bincount(eid, minlength=E)
    offs = np.concatenate([[0], np.cumsum(cnt)])
    Ce = int(max(np.ceil(cnt.max() / P) * P, P))
    CT = EPC * Ce
    nc = _get(Ce)

    Xb = np.asarray(flat_h).astype(NP_BF16)
    wgT = gate_weight.transpose(0, 2, 1).astype(NP_BF16)  # (E, K, N)
    wuT = up_weight.transpose(0, 2, 1).astype(NP_BF16)    # (E, K, N)
    wdT = down_weight.transpose(0, 2, 1).astype(NP_BF16)  # (E, N, K)
    wgT = np.ascontiguousarray(wgT)
    wuT = np.ascontiguousarray(wuT)
    wdT = np.ascontiguousarray(wdT)

    colmap = np.zeros(M * TOPK, dtype=np.int64)
    in_maps = []
    for c in range(NCORES):
        xt = np.zeros((K, CT), dtype=NP_BF16)
        gvc = np.zeros((CT,), dtype=np.float32)
        for j in range(EPC):
            e = EPC * c + j
            rows = perm[offs[e]:offs[e + 1]]
            n_e = len(rows)
            xt[:, j * Ce: j * Ce + n_e] = Xb[rows // TOPK].T
            gvc[j * Ce: j * Ce + n_e] = gvals[rows]
            colmap[rows] = c * CT + j * Ce + np.arange(n_e)
        in_maps.append(
            {
                "xt": xt,
                "wg": np.ascontiguousarray(wgT[EPC * c: EPC * (c + 1)]),
                "wu": np.ascontiguousarray(wuT[EPC * c: EPC * (c + 1)]),
                "wd": np.ascontiguousarray(wdT[EPC * c: EPC * (c + 1)]),
                "gv": gvc,
            }
        )

    res = bass_utils.run_bass_kernel_spmd(
        nc, in_maps, core_ids=list(range(NCORES)), trace=TRACE
    )
    LAST_RESULT = res
    Y = np.concatenate([np.asarray(res.results[c]["out"]) for c in range(NCORES)], axis=0)
    out = Y[colmap[0::2]] + Y[colmap[1::2]]
    return np.ascontiguousarray(out, dtype=np.float32)


# revision 25
# speedup vs baseline: 1.1829x; 1.0576x over previous
"""Grouped SwiGLU MoE (M=8192, K=2048, N=1024, E=16, top-2) on 8 TRN2 cores.

Strategy: expert-parallel. Host sorts the M*top_k expanded token slots by
expert, gathers+transposes the activations per core (core c owns experts
2c, 2c+1; each expert segment is zero-padded to a static capacity Ce),
and pre-transposes the three weight tensors to bf16. Each core runs a
dense per-expert GEMM chain:

    H^T[n, m] = silu(Wg^T-contract) * (Wu^T-contract)     (PSUM f32, bf16 out)
    out[m, k] = gate_m * (H^T.T @ Wd^T)                   (per-row scale fused)

The host combine is two pure row-gathers + one add (each token appears in
exactly top_k=2 expanded slots).
"""

import numpy as np
import ml_dtypes

import concourse.bass as bass  # noqa: F401  (engine namespace comes via nc)
import concourse.mybir as mybir
import concourse.tile as tile
from concourse import bacc, bass_utils

M, K, N, E, TOPK = 8192, 2048, 1024, 16, 2
NCORES = 8
EPC = E // NCORES  # experts per core
P = 128
KT = K // P   # 16 k-tiles
NT = N // P   # 8 n-tiles
KKC = K // 512  # 4 output column chunks

BF16 = mybir.dt.bfloat16
F32 = mybir.dt.float32
NP_BF16 = ml_dtypes.bfloat16

# Set by a driving harness to collect a profile; read back via LAST_RESULT.
TRACE = False
LAST_RESULT = None

_compiled = {}


def _chunks(Ce):
    out = []
    m0 = 0
    while m0 < Ce:
        f = min(512, Ce - m0)
        out.append((m0, f))
        m0 += f
    return out


def _build(caps):
    """caps: per-segment column capacities (seg j of every core holds one
    expert, zero-padded to caps[j]). Sorted pairing on the host means
    caps[0] covers only the small half of the expert-count distribution."""
    CT = sum(caps)
    seg_off = [0]
    for c in caps[:-1]:
        seg_off.append(seg_off[-1] + c)
    nc = bacc.Bacc()
    xt = nc.dram_tensor("xt", [K, CT], BF16, kind="ExternalInput")
    wg = nc.dram_tensor("wg", [EPC, K, N], BF16, kind="ExternalInput")
    wu = nc.dram_tensor("wu", [EPC, K, N], BF16, kind="ExternalInput")
    wd = nc.dram_tensor("wd", [EPC, N, K], BF16, kind="ExternalInput")
    gv = nc.dram_tensor("gv", [CT], F32, kind="ExternalInput")
    out = nc.dram_tensor("out", [CT, K], F32, kind="ExternalOutput")

    xt_p = xt.rearrange("(kt p) c -> p kt c", p=P)      # [128, KT, CT]
    gv_p = gv.rearrange("(t p) -> p t", p=P)            # [128, CT//P]

    with tile.TileContext(nc) as tc:
        with (
            tc.tile_pool(name="wpool", bufs=1) as wpool,
            tc.tile_pool(name="xpool", bufs=2) as xpool,
            tc.tile_pool(name="hpool", bufs=3) as hpool,
            tc.tile_pool(name="spool", bufs=8) as spool,
            tc.tile_pool(name="opool", bufs=2) as opool,
            tc.tile_pool(name="gpool", bufs=1) as gpool,
            tc.tile_pool(name="psum", bufs=8, space="PSUM") as psum,
        ):

            def g1g2(wg_sb, wu_sb, xt_sb, F, kt_outer):
                """Compute H^T for one m-chunk; returns the bf16 ht tile."""
                ht_sb = hpool.tile([P, NT * F], BF16, tag="ht")
                if kt_outer:
                    # All NT accumulation groups open at once so the PE can
                    # consume each wg/xt k-block the moment its DMA lands
                    # (startup: weights are still streaming in from HBM).
                    pgs = [psum.tile([P, F], F32, tag="ps", name=f"pg{nt}") for nt in range(NT)]
                    # Warm the PE HAM clock gate during the initial weight
                    # stream: ~3.4us of junk matmuls on a memset scratch tile
                    # so the first real groups run at 2.4 GHz, not 1.2.
                    scr = xpool.tile([P, P], BF16, tag="scr", name="scr")
                    nc.vector.memset(scr[:], 0.0)
                    for i in range(32):
                        nc.tensor.matmul(
                            pgs[0][:, :P],
                            scr[:],
                            scr[:],
                            start=(i == 0),
                            stop=(i == 31),
                        )
                    for kt in range(KT):
                        for nt in range(NT):
                            nc.tensor.matmul(
                                pgs[nt][:],
                                wg_sb[:, kt * N + nt * P: kt * N + nt * P + P],
                                xt_sb[:, kt * F: (kt + 1) * F],
                                start=(kt == 0),
                                stop=(kt == KT - 1),
                            )
                    hgs = []
                    for nt in range(NT):
                        hg = spool.tile([P, F], F32, tag="hg")
                        nc.scalar.activation(
                            hg[:], pgs[nt][:], mybir.ActivationFunctionType.Silu
                        )
                        hgs.append(hg)
                    pus = [psum.tile([P, F], F32, tag="ps", name=f"pu{nt}") for nt in range(NT)]
                    for kt in range(KT):
                        for nt in range(NT):
                            nc.tensor.matmul(
                                pus[nt][:],
                                wu_sb[:, kt * N + nt * P: kt * N + nt * P + P],
                                xt_sb[:, kt * F: (kt + 1) * F],
                                start=(kt == 0),
                                stop=(kt == KT - 1),
                            )
                    for nt in range(NT):
                        nc.vector.tensor_tensor(
                            ht_sb[:, nt * F: (nt + 1) * F],
                            hgs[nt][:],
                            pus[nt][:],
                            mybir.AluOpType.mult,
                        )
                else:
                    for nt in range(NT):
                        pg = psum.tile([P, F], F32, tag="ps")
                        pu = psum.tile([P, F], F32, tag="ps")
                        for kt in range(KT):
                            nc.tensor.matmul(
                                pg[:],
                                wg_sb[:, kt * N + nt * P: kt * N + nt * P + P],
                                xt_sb[:, kt * F: (kt + 1) * F],
                                start=(kt == 0),
                                stop=(kt == KT - 1),
                            )
                        for kt in range(KT):
                            nc.tensor.matmul(
                                pu[:],
                                wu_sb[:, kt * N + nt * P: kt * N + nt * P + P],
                                xt_sb[:, kt * F: (kt + 1) * F],
                                start=(kt == 0),
                                stop=(kt == KT - 1),
                            )
                        hg = spool.tile([P, F], F32, tag="hg")
                        nc.scalar.activation(
                            hg[:], pg[:], mybir.ActivationFunctionType.Silu
                        )
                        nc.vector.tensor_tensor(
                            ht_sb[:, nt * F: (nt + 1) * F],
                            hg[:],
                            pu[:],
                            mybir.AluOpType.mult,
                        )
                return ht_sb

            def g3(wd_sb, ht_sb, F, col0):
                for ms in range(F // P):
                    tidx = (col0 + ms * P) // P
                    ot = opool.tile([P, K], F32, tag="ot")
                    for kc in range(KKC):
                        po = psum.tile([P, 512], F32, tag="ps")
                        for nt in range(NT):
                            nc.tensor.matmul(
                                po[:],
                                ht_sb[:, nt * F + ms * P: nt * F + ms * P + P],
                                wd_sb[:, nt * K + kc * 512: nt * K + (kc + 1) * 512],
                                start=(nt == 0),
                                stop=(nt == NT - 1),
                            )
                        nc.vector.tensor_scalar_mul(
                            ot[:, kc * 512:(kc + 1) * 512],
                            po[:],
                            gv_sb[:, tidx:tidx + 1],
                        )
                        nc.sync.dma_start(
                            out=out[col0 + ms * P: col0 + ms * P + P,
                                    kc * 512:(kc + 1) * 512],
                            in_=ot[:, kc * 512:(kc + 1) * 512],
                        )

            gv_sb = gpool.tile([P, CT // P], F32, tag="gv")
            # G3 is deferred one chunk (software pipeline): its matmuls fill
            # the PE bubble while the next chunk's silu/mul chain drains, and
            # the next expert's weight DMAs hide under the deferred G3 work.
            pending = None
            for e in range(EPC):
                chunks = _chunks(caps[e])
                wg_sb = wpool.tile([P, KT * N], BF16, tag="wg")
                wu_sb = wpool.tile([P, KT * N], BF16, tag="wu")
                wd_sb = wpool.tile([P, NT * K], BF16, tag="wd")
                wg_d = wg[e].rearrange("(kt p) n -> p kt n", p=P)
                wu_d = wu[e].rearrange("(kt p) n -> p kt n", p=P)
                wd_d = wd[e].rearrange("(nt p) k -> p nt k", p=P)
                # First chunk's activations interleave with wg so the first
                # GEMM group can start as soon as the k-blocks land; wu/wd
                # stream behind it under the shadow of G1/G2 compute.
                F0 = chunks[0][1]
                xt0_sb = xpool.tile([P, KT * F0], BF16, tag="xt")
                for kt in range(KT):
                    nc.sync.dma_start(
                        out=wg_sb[:, kt * N:(kt + 1) * N], in_=wg_d[:, kt, :]
                    )
                    nc.sync.dma_start(
                        out=xt0_sb[:, kt * F0:(kt + 1) * F0],
                        in_=xt_p[:, kt, seg_off[e]:seg_off[e] + F0],
                    )
                for kt in range(KT):
                    nc.sync.dma_start(
                        out=wu_sb[:, kt * N:(kt + 1) * N], in_=wu_d[:, kt, :]
                    )
                for nt in range(NT):
                    nc.sync.dma_start(
                        out=wd_sb[:, nt * K:(nt + 1) * K], in_=wd_d[:, nt, :]
                    )
                if e == 0:
                    # Scattered 4-byte gather; emitted after the bulk weight
                    # DMAs so it can't head-of-line block them. First needed
                    # by the (deferred) first G3.
                    nc.sync.dma_start(out=gv_sb[:], in_=gv_p[:])
                if pending is not None:
                    # Flush after this expert's weight-DMA emission (so the
                    # transfers start under this G3's compute cover) but
                    # before its first GEMM group: the PE is in-order, and
                    # this G3's work is ready now while the new expert's
                    # matmuls would head-of-line block on the weight DMAs.
                    g3(*pending)
                    pending = None
                for ci, (m0, F) in enumerate(chunks):
                    col0 = seg_off[e] + m0
                    if ci == 0:
                        xt_sb = xt0_sb
                    else:
                        xt_sb = xpool.tile([P, KT * F], BF16, tag="xt")
                        for kt in range(KT):
                            nc.sync.dma_start(
                                out=xt_sb[:, kt * F:(kt + 1) * F],
                                in_=xt_p[:, kt, col0:col0 + F],
                            )
                    ht_sb = g1g2(wg_sb, wu_sb, xt_sb, F, kt_outer=(e == 0 and ci == 0))
                    if pending is not None:
                        g3(*pending)
                    pending = (wd_sb, ht_sb, F, col0)
            if pending is not None:
                g3(*pending)
    nc.compile()
    return nc


def _get(caps):
    if caps not in _compiled:
        _compiled[caps] = _build(caps)
    return _compiled[caps]


def kernel(flat_h, flat_idx, flat_gate, gate_weight, up_weight, down_weight):
    global LAST_RESULT
    eid = np.asarray(flat_idx).reshape(-1).astype(np.int64)
    gvals = np.asarray(flat_gate).reshape(-1).astype(np.float32)
    perm = np.argsort(eid, kind="stable")
    cnt = np.bincount(eid, minlength=E)
    offs = np.concatenate([[0], np.cumsum(cnt)])
    # Sorted pairing: core c gets the c-th smallest (segment 0) and c-th
    # largest (segment 1) expert, so segment 0's capacity only covers the
    # small half of the count distribution instead of the global max.
    order = np.argsort(-cnt, kind="stable")
    Cb = int(max(np.ceil(cnt[order[0]] / P) * P, P))
    Cs = int(max(np.ceil(cnt[order[NCORES]] / P) * P, P))
    caps = (Cs, Cb)
    CT = Cs + Cb
    seg_off = [0, Cs]
    nc = _get(caps)

    Xb = np.asarray(flat_h).astype(NP_BF16)
    wgT = gate_weight.transpose(0, 2, 1).astype(NP_BF16)  # (E, K, N)
    wuT = up_weight.transpose(0, 2, 1).astype(NP_BF16)    # (E, K, N)
    wdT = down_weight.transpose(0, 2, 1).astype(NP_BF16)  # (E, N, K)
    wgT = np.ascontiguousarray(wgT)
    wuT = np.ascontiguousarray(wuT)
    wdT = np.ascontiguousarray(wdT)

    colmap = np.zeros(M * TOPK, dtype=np.int64)
    in_maps = []
    for c in range(NCORES):
        xt = np.zeros((K, CT), dtype=NP_BF16)
        gvc = np.zeros((CT,), dtype=np.float32)
        exs = [int(order[2 * NCORES - 1 - c]), int(order[c])]
        for j, e in enumerate(exs):
            rows = perm[offs[e]:offs[e + 1]]
            n_e = len(rows)
            xt[:, seg_off[j]: seg_off[j] + n_e] = Xb[rows // TOPK].T
            gvc[seg_off[j]: seg_off[j] + n_e] = gvals[rows]
            colmap[rows] = c * CT + seg_off[j] + np.arange(n_e)
        in_maps.append(
            {
                "xt": xt,
                "wg": np.ascontiguousarray(np.stack([wgT[e] for e in exs])),
                "wu": np.ascontiguousarray(np.stack([wuT[e] for e in exs])),
                "wd": np.ascontiguousarray(np.stack([wdT[e] for e in exs])),
                "gv": gvc,
            }
        )

    res = bass_utils.run_bass_kernel_spmd(
        nc, in_maps, core_ids=list(range(NCORES)), trace=TRACE
    )
    LAST_RESULT = res
    Y = np.concatenate([np.asarray(res.results[c]["out"]) for c in range(NCORES)], axis=0)
    out = Y[colmap[0::2]] + Y[colmap[1::2]]
    return np.ascontiguousarray(out, dtype=np.float32)
